# revision 12
# baseline (speedup 1.0000x reference)
"""CompGCN message-passing kernel for 8 Trainium2 NeuronCores.

Strategy (edges sharded by dst range; per-core node range = N/8):
  Algebra: h[dst]*e_h segment-sum collapses to h * segsum(e_h,dst);
  text_linear folds through the segment sum (linearity) and into W_rel.
  Launch 1 (per core): stream sorted+padded edge features; selector-matmul
    segment sums of [e_h|s_h] by dst into PSUM per 128-node block; node phase
    produces h_s_r_o block table (bf16) + full e_h_new^T via weight matmuls.
  Host: assemble full h_s_r_o table (25.6 -> 12.8MB bf16), replicate.
  Launch 2 (per core): dma_gather table rows by src (parity-split int16
    addressing), selector-matmul segment sum by dst, *norm + bias_v -> h_new.
"""
import sys

sys.path.insert(0, "/opt/trn_rl_repo")

import numpy as np
from ml_dtypes import bfloat16

import concourse.bass as bass
import concourse.mybir as mybir
from concourse.tile import TileContext
from concourse import bass_utils

# ---------------------------------------------------------------- tilefix ---
# This container's walrus supports a single sync-wait command per instruction.
# Rewrite the serialized BIR before walrus: any instruction with >1 wait gets
# preceded by single-wait NoOps on the same engine (same-queue order preserved).
import json as _json

import concourse.bass2jax as _b2j
from concourse.bass_utils import compile_bir_kernel as _orig_cbk

_WSP = [0]


def _split_bir_waits(bir_bytes):
    d = _json.loads(bir_bytes)
    for fn in d["functions"]:
        for bb in fn["blocks"]:
            out = []
            for ins in bb["instructions"]:
                si = ins.get("sync_info")
                ow = (si or {}).get("on_wait") or []
                if len(ow) > 1:
                    for w in ow[:-1]:
                        _WSP[0] += 1
                        out.append({
                            "debug": ins.get("debug", 0),
                            "engine": ins["engine"], "ins": [],
                            "name": f"WSP-{_WSP[0]}", "opcode": "NoOp",
                            "outs": [],
                            "sync_info": {"on_update": [], "on_wait": [w]},
                        })
                    si["on_wait"] = [ow[-1]]
                out.append(ins)
            bb["instructions"] = out
    return _json.dumps(d).encode()


def _patched_cbk(bir_str, compile_dir, neff_name="file.neff", **kw):
    return _orig_cbk(_split_bir_waits(bir_str), compile_dir,
                     neff_name=neff_name, **kw)


_b2j.compile_bir_kernel = _patched_cbk

# ---------------------------------------------------------------- problem ---
N, E = 50000, 800000
D = 128
NCORES = 8
NPC = N // NCORES            # 6250 nodes per core
NB = (NPC + 127) // 128      # 49 blocks of 128 nodes (last partial)
NPAD = NB * 128              # 6272
F32 = mybir.dt.float32
BF16 = mybir.dt.bfloat16
I16 = mybir.dt.int16


def _build_launch1(Kh, EPW):
    """Phase 1 (segment sums + node phase -> hsro table shard) and the
    independent e_h_new weight pass (outputs e_h_new^T)."""
    G = 2 * NB * Kh          # edge groups of 128 per core
    nc = bass.Bass(target_bir_lowering=False)
    cat = nc.dram_tensor("cat", [2 * NB, 128, Kh * 256], BF16, kind="ExternalInput")
    catT = nc.dram_tensor("catT", [256, EPW], BF16, kind="ExternalInput")
    dstrel = nc.dram_tensor("dstrel", [128, G], F32, kind="ExternalInput")
    hresh = nc.dram_tensor("hresh", [128, NB * 128], F32, kind="ExternalInput")
    denomr = nc.dram_tensor("denomr", [128, NB], F32, kind="ExternalInput")
    indrow = nc.dram_tensor("indrow", [1, NB * 128], F32, kind="ExternalInput")
    iota = nc.dram_tensor("iota", [128, 128], BF16, kind="ExternalInput")
    ident = nc.dram_tensor("ident", [128, 128], F32, kind="ExternalInput")
    wsi = nc.dram_tensor("wsi", [128, 128], F32, kind="ExternalInput")
    wbot = nc.dram_tensor("wbot", [128, 128], F32, kind="ExternalInput")
    bvec2 = nc.dram_tensor("bvec2", [1, 128], F32, kind="ExternalInput")
    binv = nc.dram_tensor("binv", [1, 128], F32, kind="ExternalInput")
    wr1 = nc.dram_tensor("wr1", [128, 128], BF16, kind="ExternalInput")
    wts = nc.dram_tensor("wts", [128, 128], BF16, kind="ExternalInput")
    bcomb = nc.dram_tensor("bcomb", [128, 1], F32, kind="ExternalInput")
    ones = nc.dram_tensor("ones", [1, 128], F32, kind="ExternalInput")
    hsro = nc.dram_tensor("hsro", [NPAD, 128], BF16, kind="ExternalOutput")
    ehnT = nc.dram_tensor("ehnT", [128, EPW], F32, kind="ExternalOutput")

    NT = EPW // 512
    with TileContext(nc) as tc:
        with (
            tc.tile_pool(name="const", bufs=1) as cpool,
            tc.tile_pool(name="edges", bufs=3) as epool,
            tc.tile_pool(name="work", bufs=3) as wpool,
            tc.tile_pool(name="ps", bufs=2, space="PSUM") as pspool,
            tc.tile_pool(name="psw", bufs=2, space="PSUM") as pswpool,
            tc.tile_pool(name="psn", bufs=2, space="PSUM") as psnpool,
        ):
            iot = cpool.tile([128, 128], BF16, tag="iota")
            nc.sync.dma_start(out=iot[:, :], in_=iota[:, :])
            idn = cpool.tile([128, 128], F32, tag="ident")
            nc.sync.dma_start(out=idn[:, :], in_=ident[:, :])
            dr = cpool.tile([128, G], F32, tag="dstrel")
            nc.sync.dma_start(out=dr[:, :], in_=dstrel[:, :])
            hr = cpool.tile([128, NB * 128], F32, tag="hresh")
            nc.sync.dma_start(out=hr[:, :], in_=hresh[:, :])
            dn = cpool.tile([128, NB], F32, tag="denomr")
            nc.sync.dma_start(out=dn[:, :], in_=denomr[:, :])
            ir = cpool.tile([1, NB * 128], F32, tag="indrow")
            nc.sync.dma_start(out=ir[:, :], in_=indrow[:, :])
            w1 = cpool.tile([128, 128], F32, tag="wsi")
            nc.sync.dma_start(out=w1[:, :], in_=wsi[:, :])
            w2 = cpool.tile([128, 128], F32, tag="wbot")
            nc.sync.dma_start(out=w2[:, :], in_=wbot[:, :])
            bv2 = cpool.tile([1, 128], F32, tag="bvec2")
            nc.sync.dma_start(out=bv2[:, :], in_=bvec2[:, :])
            bi = cpool.tile([1, 128], F32, tag="binv")
            nc.sync.dma_start(out=bi[:, :], in_=binv[:, :])
            one_row = cpool.tile([1, 128], F32, tag="ones")
            nc.sync.dma_start(out=one_row[:, :], in_=ones[:, :])
            wa = cpool.tile([128, 128], BF16, tag="wr1")
            nc.sync.dma_start(out=wa[:, :], in_=wr1[:, :])
            wb = cpool.tile([128, 128], BF16, tag="wts")
            nc.sync.dma_start(out=wb[:, :], in_=wts[:, :])
            bc = cpool.tile([128, 1], F32, tag="bcomb")
            nc.sync.dma_start(out=bc[:, :], in_=bcomb[:, :])

            # ---- weight pass: e_h_new^T = (Wr1^T @ e_h^T + Wts^T @ s_h^T) + b
            for t in range(NT):
                ea = epool.tile([128, 512], BF16, tag="wp_e")
                nc.sync.dma_start(out=ea[:, :], in_=catT[0:128, t * 512:(t + 1) * 512])
                sa = epool.tile([128, 512], BF16, tag="wp_s")
                nc.sync.dma_start(out=sa[:, :], in_=catT[128:256, t * 512:(t + 1) * 512])
                pw = pswpool.tile([128, 512], F32, tag="wp_ps")
                nc.tensor.matmul(pw[:, :], wa[:, :], ea[:, :], start=True, stop=False)
                nc.tensor.matmul(pw[:, :], wb[:, :], sa[:, :], start=False, stop=True)
                ob = wpool.tile([128, 512], F32, tag="wp_out")
                nc.vector.tensor_scalar(
                    ob[:, :], pw[:, :], bc[:, 0:1], None, mybir.AluOpType.add
                )
                nc.sync.dma_start(out=ehnT[:, t * 512:(t + 1) * 512], in_=ob[:, :])

            # ---- phase 1 + node phase, per 128-node block
            for b in range(NB):
                pseg = pspool.tile([128, 256], F32, tag="seg")
                for half in range(2):
                    hb = 2 * b + half
                    et = epool.tile([128, Kh * 256], BF16, tag="p1_edges")
                    nc.sync.dma_start(out=et[:, :], in_=cat[hb, :, :])
                    for k in range(Kh):
                        g = hb * Kh + k
                        S = wpool.tile([128, 128], BF16, tag="sel")
                        nc.vector.tensor_scalar(
                            S[:, :], iot[:, :], dr[:, g:g + 1], None,
                            mybir.AluOpType.is_equal,
                        )
                        nc.tensor.matmul(
                            pseg[:, :], S[:, :], et[:, k * 256:(k + 1) * 256],
                            start=(half == 0 and k == 0),
                            stop=(half == 1 and k == Kh - 1),
                        )
                # node phase for block b  (A=pseg[:, :128], B=pseg[:,128:])
                bp = wpool.tile([128, 128], F32, tag="bprime")
                nc.vector.tensor_scalar(
                    bp[:, :], pseg[:, 128:256], dn[:, b:b + 1], None,
                    mybir.AluOpType.mult,
                )
                x1 = wpool.tile([128, 128], F32, tag="x1")
                nc.vector.tensor_tensor(
                    x1[:, :], pseg[:, 0:128], hr[:, b * 128:(b + 1) * 128],
                    mybir.AluOpType.mult,
                )
                nc.vector.tensor_scalar(
                    x1[:, :], x1[:, :], dn[:, b:b + 1], None, mybir.AluOpType.mult
                )
                ptr = psnpool.tile([128, 128], F32, tag="tp")
                nc.tensor.transpose(ptr[:, :], bp[:, :], idn[:, :])
                t1 = wpool.tile([128, 128], F32, tag="t1")
                nc.vector.tensor_copy(t1[:, :], ptr[:, :])
                ptr2 = psnpool.tile([128, 128], F32, tag="tp")
                nc.tensor.transpose(ptr2[:, :], x1[:, :], idn[:, :])
                t2 = wpool.tile([128, 128], F32, tag="t2")
                nc.vector.tensor_copy(t2[:, :], ptr2[:, :])
                p2 = psnpool.tile([128, 128], F32, tag="hsro_ps")
                nc.tensor.matmul(p2[:, :], t1[:, :], w1[:, :], start=True, stop=False)
                nc.tensor.matmul(p2[:, :], t2[:, :], w2[:, :], start=False, stop=False)
                nc.tensor.matmul(
                    p2[:, :], ir[0:1, b * 128:(b + 1) * 128], bv2[0:1, :],
                    start=False, stop=False,
                )
                nc.tensor.matmul(
                    p2[:, :], one_row[0:1, :], bi[0:1, :], start=False, stop=True
                )
                tb = wpool.tile([128, 128], BF16, tag="tab")
                nc.vector.tensor_copy(tb[:, :], p2[:, :])
                nc.sync.dma_start(out=hsro[b * 128:(b + 1) * 128, :], in_=tb[:, :])
    return nc


def _build_launch2(Kh):
    """Phase 2: h_new = segsum(table[src], dst) * norm + bias_v.
    Gathered messages (table[src]) are staged on host into gcat."""
    G = 2 * NB * Kh
    nc = bass.Bass(target_bir_lowering=False)
    gcat = nc.dram_tensor("gcat", [2 * NB, 128, Kh * 128], BF16,
                          kind="ExternalInput")
    dstrel = nc.dram_tensor("dstrel", [128, G], F32, kind="ExternalInput")
    normr = nc.dram_tensor("normr", [128, NB], F32, kind="ExternalInput")
    biasv = nc.dram_tensor("biasv", [128, 128], F32, kind="ExternalInput")
    iota = nc.dram_tensor("iota", [128, 128], BF16, kind="ExternalInput")
    hnew = nc.dram_tensor("hnew", [NPAD, 128], F32, kind="ExternalOutput")

    with TileContext(nc) as tc:
        with (
            tc.tile_pool(name="const", bufs=1) as cpool,
            tc.tile_pool(name="work", bufs=3) as wpool,
            tc.tile_pool(name="gath", bufs=3) as gpool,
            tc.tile_pool(name="ps", bufs=2, space="PSUM") as pspool,
        ):
            iot = cpool.tile([128, 128], BF16, tag="iota")
            nc.sync.dma_start(out=iot[:, :], in_=iota[:, :])
            dr = cpool.tile([128, G], F32, tag="dstrel")
            nc.sync.dma_start(out=dr[:, :], in_=dstrel[:, :])
            nr = cpool.tile([128, NB], F32, tag="normr")
            nc.sync.dma_start(out=nr[:, :], in_=normr[:, :])
            bv = cpool.tile([128, 128], F32, tag="biasv")
            nc.sync.dma_start(out=bv[:, :], in_=biasv[:, :])

            for b in range(NB):
                pseg = pspool.tile([128, 128], F32, tag="seg")
                for half in range(2):
                    hb = 2 * b + half
                    gt = gpool.tile([128, Kh * 128], BF16, tag="gath")
                    nc.sync.dma_start(out=gt[:, :], in_=gcat[hb, :, :])
                    for k in range(Kh):
                        g = hb * Kh + k
                        S = wpool.tile([128, 128], BF16, tag="sel")
                        nc.vector.tensor_scalar(
                            S[:, :], iot[:, :], dr[:, g:g + 1], None,
                            mybir.AluOpType.is_equal,
                        )
                        nc.tensor.matmul(
                            pseg[:, :], S[:, :], gt[:, k * 128:(k + 1) * 128],
                            start=(half == 0 and k == 0),
                            stop=(half == 1 and k == Kh - 1),
                        )
                hn = wpool.tile([128, 128], F32, tag="hn")
                nc.vector.tensor_scalar(
                    hn[:, :], pseg[:, :], nr[:, b:b + 1], None, mybir.AluOpType.mult
                )
                nc.vector.tensor_tensor(
                    hn[:, :], hn[:, :], bv[:, :], mybir.AluOpType.add
                )
                nc.sync.dma_start(out=hnew[b * 128:(b + 1) * 128, :], in_=hn[:, :])
    return nc


def _prep(h, e_h, s_h, norm, src, dst, W_text, b_text, W_inv, b_inv, W_rel,
          b_rel, bias_v):
    """Host-side sharding/layout. Returns launch input maps + assembly info."""
    src = np.asarray(src).astype(np.int64)
    dst = np.asarray(dst).astype(np.int64)
    h = np.asarray(h, np.float32)
    e_h = np.asarray(e_h, np.float32)
    s_h = np.asarray(s_h, np.float32)
    norm = np.asarray(norm, np.float32)

    core = dst // NPC
    par = (src & 1).astype(np.int64)
    blk = (dst - core * NPC) // 128
    key = (core * NB + blk) * 2 + par                      # global half-block id
    nkey = NCORES * NB * 2
    cnts = np.bincount(key, minlength=nkey)
    Kh = max(1, int(np.ceil(cnts.max() / 128.0)))
    EPC = 2 * NB * Kh * 128                                # padded edges/core
    order2 = np.argsort(key, kind="stable")
    starts = np.zeros(nkey, np.int64)
    starts[1:] = np.cumsum(cnts)[:-1]
    rank = np.empty(E, np.int64)
    rank[order2] = np.arange(E) - np.repeat(starts, cnts)
    ppg = key * (Kh * 128) + rank                          # global padded pos

    EPALL = NCORES * EPC
    catp = np.zeros((EPALL, 256), dtype=bfloat16)
    catp[ppg, :128] = e_h.astype(bfloat16)
    catp[ppg, 128:] = s_h.astype(bfloat16)
    drel = np.full(EPALL, -1.0, np.float32)
    drel[ppg] = (dst - core * NPC - blk * 128).astype(np.float32)
    srcfull = np.zeros(EPALL, np.int64)
    srcfull[ppg] = src

    deg = np.bincount(dst, minlength=N).astype(np.float32)
    denom = np.maximum(deg, 1.0)
    dr_full = (1.0 / denom)
    ind_full = (deg > 0).astype(np.float32)

    W_top = np.asarray(W_inv, np.float32)[:128]
    W_bot = np.asarray(W_inv, np.float32)[128:]
    W_text = np.asarray(W_text, np.float32)
    W_rel = np.asarray(W_rel, np.float32)
    wsi = (W_text @ W_top).astype(np.float32)
    bvec2 = (np.asarray(b_text, np.float32) @ W_top).reshape(1, 128)
    binv = np.asarray(b_inv, np.float32).reshape(1, 128)
    wr1 = W_rel[:128].astype(bfloat16)
    wts = (W_text @ W_rel[128:]).astype(bfloat16)
    bcomb = (np.asarray(b_text, np.float32) @ W_rel[128:]
             + np.asarray(b_rel, np.float32)).reshape(128, 1).astype(np.float32)
    iota_bf = np.broadcast_to(
        np.arange(128, dtype=np.float32), (128, 128)
    ).astype(bfloat16).copy()
    ident = np.eye(128, dtype=np.float32)
    biasv_t = np.broadcast_to(
        np.asarray(bias_v, np.float32), (128, 128)
    ).copy()

    EPW = ((EPC + 511) // 512) * 512
    in1, in2, src_slices = [], [], []
    for c in range(NCORES):
        cp = catp[c * EPC:(c + 1) * EPC]
        cat_c = (cp.reshape(2 * NB, Kh, 128, 256)
                 .transpose(0, 2, 1, 3).reshape(2 * NB, 128, Kh * 256).copy())
        catT_c = np.zeros((256, EPW), dtype=bfloat16)
        catT_c[:, :EPC] = cp.T
        drel_c = np.ascontiguousarray(
            drel[c * EPC:(c + 1) * EPC].reshape(2 * NB * Kh, 128).T)
        srcc = srcfull[c * EPC:(c + 1) * EPC]
        hp = np.zeros((NPAD, 128), np.float32)
        hp[:NPC] = h[c * NPC:(c + 1) * NPC]
        hresh_c = hp.reshape(NB, 128, 128).transpose(1, 0, 2).reshape(
            128, NB * 128).copy()
        dnp = np.ones(NPAD, np.float32)
        dnp[:NPC] = dr_full[c * NPC:(c + 1) * NPC]
        denomr_c = dnp.reshape(NB, 128).T.copy()
        inp_ = np.zeros(NPAD, np.float32)
        inp_[:NPC] = ind_full[c * NPC:(c + 1) * NPC]
        indrow_c = inp_.reshape(1, NPAD)
        nrp = np.zeros(NPAD, np.float32)
        nrp[:NPC] = norm[c * NPC:(c + 1) * NPC, 0]
        normr_c = nrp.reshape(NB, 128).T.copy()
        in1.append(dict(
            cat=cat_c, catT=catT_c, dstrel=drel_c, hresh=hresh_c,
            denomr=denomr_c, indrow=indrow_c, iota=iota_bf, ident=ident,
            wsi=wsi, wbot=W_bot.astype(np.float32), bvec2=bvec2, binv=binv,
            wr1=wr1, wts=wts, bcomb=bcomb, ones=np.ones((1, 128), np.float32),
        ))
        in2.append(dict(
            dstrel=drel_c, normr=normr_c, biasv=biasv_t, iota=iota_bf,
        ))
        src_slices.append(srcc)
    return Kh, EPW, EPC, in1, in2, ppg, core, src_slices


def kernel(h, e_h, s_h, norm, src, dst, W_text, b_text, W_inv, b_inv, W_rel,
           b_rel, bias_v, _profile=None):
    Kh, EPW, EPC, in1, in2, ppg, core_of, src_slices = _prep(
        h, e_h, s_h, norm, src, dst, W_text, b_text, W_inv, b_inv, W_rel,
        b_rel, bias_v)

    import time as _time
    nc1 = _build_launch1(Kh, EPW)
    t0 = _time.time()
    res1 = bass_utils.run_bass_kernel_spmd(
        nc1, in1, core_ids=list(range(NCORES)))
    t1 = _time.time()
    if _profile is not None:
        # cache-warm re-exec: jax compilation cache hits on identical HLO,
        # so this times upload+exec+download only
        res1 = bass_utils.run_bass_kernel_spmd(
            nc1, in1, core_ids=list(range(NCORES)))
        _profile["l1_warm_s"] = _time.time() - t1
        _profile["l1_cold_s"] = t1 - t0

    # assemble full bf16 table, replicate to all cores
    table = np.concatenate(
        [res1.results[c]["hsro"][:NPC] for c in range(NCORES)], axis=0)
    for c in range(NCORES):
        gc = table[src_slices[c]]
        in2[c]["gcat"] = np.ascontiguousarray(
            gc.reshape(2 * NB, Kh, 128, 128).transpose(0, 2, 1, 3)
            .reshape(2 * NB, 128, Kh * 128))

    nc2 = _build_launch2(Kh)
    t2 = _time.time()
    res2 = bass_utils.run_bass_kernel_spmd(
        nc2, in2, core_ids=list(range(NCORES)))
    t3 = _time.time()
    if _profile is not None:
        res2 = bass_utils.run_bass_kernel_spmd(
            nc2, in2, core_ids=list(range(NCORES)))
        _profile["l2_warm_s"] = _time.time() - t3
        _profile["l2_cold_s"] = t3 - t2

    # h_new
    h_new = np.concatenate(
        [res2.results[c]["hnew"][:NPC] for c in range(NCORES)], axis=0)
    h_new = np.ascontiguousarray(h_new, dtype=np.float32)

    # e_h_new: un-transpose + un-pad + un-permute
    e_h_new = np.empty((E, 128), np.float32)
    pos_in_core = ppg - core_of * EPC
    for c in range(NCORES):
        m = core_of == c
        e_h_new[m] = res1.results[c]["ehnT"].T[pos_in_core[m]]

    if _profile is not None:
        _profile["res1"] = res1
        _profile["res2"] = res2
    return h_new, e_h_new
